# revision 27
# baseline (speedup 1.0000x reference)
"""Trainium2 Bass kernel for a full transformer block (nn_Attention_32873679684330).

Sharding: data-parallel over batch — B=8 batch elements, one per NeuronCore.
Each core runs the full block (LN1 -> QKV -> attention -> out-proj+residual ->
LN2 -> GELU MLP -> residual) on its [1024, 1024] slice, fully on-chip.

Layout: activations feature-major ([features(partitions), tokens(free)]).
Per-token statistics via ones-vector matmuls; broadcasts via K=1 outer products.

Perf strategy vs baseline:
- fp8e4m3 DoubleRow matmuls (2 contraction rows/PE-cycle) for QKV, probs@V,
  out-proj and both FFN matmuls. Weights host-scaled by powers of 2 into the
  e4m3 sweet spot; rescale folded into the PSUM read-out op.
- attention scores: the two heads of a feature chunk sit in partition rows
  0-63 / 64-127, so their K=64 matmuls row-tile onto disjoint PE quadrants and
  run concurrently (~2x).
- softmax exp on ACT writes fp8 probs with bias -2*ln2 (keeps e4m3 < 240; the
  1/4 factor cancels between numerator and ones-column denominator).
- LN scale-bias moved from ACT to DVE tensor_scalar; out-proj residual uses a
  single scalar_tensor_tensor (scale + residual add).
- emission order software-pipelines: q/k chunks for pairs 4-7 fill PE during
  softmax of pairs 0-3; out-proj of token-half 0 fills PE during attention of
  half 1; FFN tail is PE-dense.
"""

import sys

for _p in ("/root/.axon_site", "/root/.axon_site/_ro/trn_rl_repo",
           "/root/.axon_site/_ro/pypackages"):
    if _p not in sys.path:
        sys.path.append(_p)

import numpy as np
from contextlib import ExitStack

import concourse.bass as bass
import concourse.bacc as bacc
import concourse.mybir as mybir
import concourse.tile as tile
from concourse.bass_utils import run_bass_kernel_spmd

F32 = mybir.dt.float32
F32R = mybir.dt.float32r
BF16 = mybir.dt.bfloat16
FP8 = mybir.dt.float8e4
FP8E5 = mybir.dt.float8e5
NP_BF16 = np.dtype(mybir.dt.np(BF16))
NP_FP8 = np.dtype(mybir.dt.np(FP8))
AF = mybir.ActivationFunctionType
ALU = mybir.AluOpType
DR = mybir.MatmulPerfMode.DoubleRow

B, P, E, H, DH, MLP = 8, 1024, 1024, 16, 64, 4096
SCALE = DH ** -0.5
NCORES = 8
EC = E // 128        # 8 feature chunks
TC = P // 128        # 8 token chunks
TN = P // 512        # 2 token 512-halves
MC = MLP // 128      # 32 mlp chunks
EP = EC // 2         # 4 contraction chunk-pairs over E
MP = MC // 2         # 16 contraction chunk-pairs over MLP
WQ, W1S, W2S, WOS = 32.0, 32.0, 64.0, 32.0   # host-side fp8 weight scales
EXPB = float(-2.0 * np.log(2.0))             # exp bias: probs scaled by 1/4


def round_fp32r(x):
    b = np.ascontiguousarray(x, dtype=np.float32).view(np.uint32)
    b = ((b.astype(np.uint64) + 0x800) & 0xFFFFF000).astype(np.uint32)
    return b.view(np.float32)


STAGE_RANK = {"ln1": 0, "qkv": 1, "attn": 2, "x2": 3, "ln2": 4, "full": 9}


def build_program(stage="full"):
    rank = STAGE_RANK[stage]
    nc = bacc.Bacc("TRN2", target_bir_lowering=False, debug=False,
                   num_devices=NCORES)

    xT_d = nc.dram_tensor("xT", [E, P], BF16, kind="ExternalInput").ap()
    wqkv_d = nc.dram_tensor("wqkv8", [E, 3 * E], FP8, kind="ExternalInput").ap()
    wo_d = nc.dram_tensor("wo8", [E, E], FP8, kind="ExternalInput").ap()
    w1_d = nc.dram_tensor("w1", [E, MLP], BF16, kind="ExternalInput").ap()
    w2_d = nc.dram_tensor("w2", [MLP, E], BF16, kind="ExternalInput").ap()
    bv_row_d = nc.dram_tensor("bv_row", [1, E], BF16, kind="ExternalInput").ap()
    bo_pm_d = nc.dram_tensor("bo_pm", [128, EC], F32, kind="ExternalInput").ap()
    b2_pm_d = nc.dram_tensor("b2_pm", [128, EC], F32, kind="ExternalInput").ap()
    bqkv_pm_d = nc.dram_tensor("bqkv_pm", [128, 16], F32, kind="ExternalInput").ap()
    b1_pm_d = nc.dram_tensor("b1_pm", [128, MC], F32, kind="ExternalInput").ap()
    g1_pm_d = nc.dram_tensor("g1_pm", [128, EC], F32, kind="ExternalInput").ap()
    bt1_pm_d = nc.dram_tensor("bt1_pm", [128, EC], F32, kind="ExternalInput").ap()
    g2_pm_d = nc.dram_tensor("g2_pm", [128, EC], F32, kind="ExternalInput").ap()
    bt2_pm_d = nc.dram_tensor("bt2_pm", [128, EC], F32, kind="ExternalInput").ap()
    ones_row_d = nc.dram_tensor("ones_row", [1, 512], BF16, kind="ExternalInput").ap()
    ones_col_d = nc.dram_tensor("ones_col", [128, 1], F32R, kind="ExternalInput").ap()
    ones_col_bf_d = nc.dram_tensor("ones_col_bf", [128, 1], BF16, kind="ExternalInput").ap()

    outT_d = nc.dram_tensor("outT", [E, P], F32, kind="ExternalOutput").ap()
    dbg_d = None
    if stage != "full":
        dbg_d = nc.dram_tensor("dbg", [4 * 1024, P], F32, kind="ExternalOutput").ap()

    with tile.TileContext(nc) as tc, ExitStack() as ctx:
        const = ctx.enter_context(tc.tile_pool(name="const", bufs=1))
        wpool = ctx.enter_context(tc.tile_pool(name="w", bufs=2))
        scr = ctx.enter_context(tc.tile_pool(name="scr", bufs=3))
        rows = ctx.enter_context(tc.tile_pool(name="rows", bufs=2))
        xTp = ctx.enter_context(tc.tile_pool(name="xTp", bufs=1))

        def cload(shape, dt, dram, cname):
            t = const.tile(shape, dt, name=cname)
            nc.sync.dma_start(t[:], dram[:])
            return t

        ones_col_bf = cload([128, 1], BF16, ones_col_bf_d, "c_ones_col_bf")
        ones_col = cload([128, 1], F32R, ones_col_d, "c_ones_col")
        ones_row = cload([1, 512], BF16, ones_row_d, "c_ones_row")

        # ---- load xT (bf16, feature-major) ----
        xT = xTp.tile([128, EC, P], BF16, tag="xT", name="xT_sb")
        for c in range(EC):
            nc.sync.dma_start(xT[:, c, :], xT_d[c * 128:(c + 1) * 128, :])

        bv_row = cload([1, E], BF16, bv_row_d, "c_bv_row")
        bo_pm = cload([128, EC], F32, bo_pm_d, "c_bo_pm")
        b2_pm = cload([128, EC], F32, b2_pm_d, "c_b2_pm")
        bqkv_pm = cload([128, 16], F32, bqkv_pm_d, "c_bqkv_pm")
        b1_pm = cload([128, MC], F32, b1_pm_d, "c_b1_pm")
        g1_pm = cload([128, EC], F32, g1_pm_d, "c_g1_pm")
        bt1_pm = cload([128, EC], F32, bt1_pm_d, "c_bt1_pm")
        g2_pm = cload([128, EC], F32, g2_pm_d, "c_g2_pm")
        bt2_pm = cload([128, EC], F32, bt2_pm_d, "c_bt2_pm")
        eps_sb = const.tile([1, 1], F32, name="c_eps")
        nc.vector.memset(eps_sb[:], 1e-5)
        expb_sb = const.tile([128, 1], F32, name="c_expb")
        nc.vector.memset(expb_sb[:], EXPB)
        warm = const.tile([1, 2], F32, name="c_warm")
        nc.vector.memset(warm[:], 1.0)
        nc.scalar.activation(warm[:, 0:1], warm[:, 1:2], AF.Exp)
        nc.scalar.activation(warm[:, 0:1], warm[:, 1:2], AF.Sqrt)

        def dump_fm(src, row0):
            dpool = tc.alloc_tile_pool(name="dump", bufs=2, side="right")
            for c in range(EC):
                st = dpool.tile([128, P], F32, tag="dump", name=f"dump_{row0}_{c}")
                nc.scalar.activation(st[:], src[:, c, :], AF.Copy)
                nc.sync.dma_start(dbg_d[row0 + c * 128: row0 + (c + 1) * 128, :], st[:])
            dpool.release()

        def ln_stats(src_t, onesc, sq_dt, nm, psLN, tn,
                     st_tag="st", st_bufs=2, bc_tag="bc", bc_bufs=2):
            """stats + bf16 SBUF broadcasts of mu and rstd for one token half."""
            sl = slice(tn * 512, (tn + 1) * 512)
            mu_ps = psLN.tile([1, 512], F32, tag=st_tag, bufs=st_bufs,
                              name=f"{nm}_mups{tn}")
            for c in range(EC):
                nc.tensor.matmul(mu_ps[:], onesc[:], src_t[:, c, sl],
                                 start=(c == 0), stop=(c == EC - 1))
            mu_row = rows.tile([1, 512], BF16, tag="mu", name=f"{nm}_mu{tn}")
            nc.scalar.activation(mu_row[:], mu_ps[:], AF.Copy, scale=1.0 / E)
            mu_f = rows.tile([1, 512], F32, tag="r", bufs=4, name=f"{nm}_muf{tn}")
            nc.scalar.activation(mu_f[:], mu_ps[:], AF.Copy, scale=1.0 / E)
            sq_ps = psLN.tile([1, 512], F32, tag=st_tag, bufs=st_bufs,
                              name=f"{nm}_sqps{tn}")
            for c in range(EC):
                sq = scr.tile([128, 512], sq_dt, tag="sq", name=f"{nm}_sq{tn}_{c}")
                nc.vector.tensor_mul(sq[:], src_t[:, c, sl], src_t[:, c, sl])
                nc.tensor.matmul(sq_ps[:], onesc[:], sq[:],
                                 start=(c == 0), stop=(c == EC - 1))
            msq = rows.tile([1, 512], F32, tag="r", bufs=4, name=f"{nm}_msq{tn}")
            nc.scalar.activation(msq[:], sq_ps[:], AF.Copy, scale=1.0 / E)
            mu2 = rows.tile([1, 512], F32, tag="r", bufs=4, name=f"{nm}_mu2{tn}")
            nc.vector.tensor_mul(mu2[:], mu_f[:], mu_f[:])
            var = rows.tile([1, 512], F32, tag="r", bufs=4, name=f"{nm}_var{tn}")
            nc.vector.tensor_sub(var[:], msq[:], mu2[:])
            sd = rows.tile([1, 512], F32, tag="r", bufs=4, name=f"{nm}_sd{tn}")
            nc.scalar.activation(sd[:], var[:], AF.Sqrt, bias=eps_sb[:])
            rf = rows.tile([1, 512], F32, tag="r", bufs=4, name=f"{nm}_rf{tn}")
            rs = rows.tile([1, 512], F32, tag="r", bufs=4, name=f"{nm}_rs{tn}")
            nc.vector.reciprocal_approx_accurate(rf[:], sd[:], rs[:])
            rstd_r = rows.tile([1, 512], BF16, tag="mu", name=f"{nm}_rstdr{tn}")
            nc.vector.tensor_copy(rstd_r[:], rf[:])
            mu_b = psLN.tile([128, 512], F32, tag=bc_tag, bufs=bc_bufs,
                             name=f"{nm}_mub{tn}")
            nc.tensor.matmul(mu_b[:], ones_row[:, :128], mu_row[:],
                             start=True, stop=True)
            r_b = psLN.tile([128, 512], F32, tag=bc_tag, bufs=bc_bufs,
                            name=f"{nm}_rb{tn}")
            nc.tensor.matmul(r_b[:], ones_row[:, :128], rstd_r[:],
                             start=True, stop=True)
            mu_bs = scr.tile([128, 512], BF16, tag="mubs", bufs=2,
                             name=f"{nm}_mubs{tn}")
            nc.vector.tensor_copy(mu_bs[:], mu_b[:])
            r_bs = scr.tile([128, 512], BF16, tag="rbs", bufs=2,
                            name=f"{nm}_rbs{tn}")
            nc.vector.tensor_copy(r_bs[:], r_b[:])
            return mu_bs, r_bs

        def ln_norm(out, src_t, g_pm, b_pm, nm, tn, mu_bs, r_bs):
            sl = slice(tn * 512, (tn + 1) * 512)
            for c in range(EC):
                d = scr.tile([128, 512], BF16, tag="lnd", bufs=2,
                             name=f"{nm}_d{tn}_{c}")
                nc.vector.tensor_sub(d[:], src_t[:, c, sl], mu_bs[:])
                e = scr.tile([128, 512], BF16, tag="lne", bufs=2,
                             name=f"{nm}_e{tn}_{c}")
                nc.vector.tensor_mul(e[:], d[:], r_bs[:])
                nc.vector.tensor_scalar(out[:, c, sl], e[:],
                                        g_pm[:, c:c + 1], b_pm[:, c:c + 1],
                                        ALU.mult, ALU.add)

        def layernorm(src_t, onesc, sq_dt, g_pm, b_pm, out_pool, out_dt, nm,
                      ps_pool=None, st_tag="st", st_bufs=2, bc_tag="bc", bc_bufs=2):
            out = out_pool.tile([128, EC, P], out_dt, tag="xn", name=f"{nm}_sb")
            psLN = ps_pool or tc.alloc_tile_pool(name=f"psLN_{nm}", bufs=2,
                                                 space="PSUM")
            bs = [ln_stats(src_t, onesc, sq_dt, nm, psLN, tn,
                           st_tag, st_bufs, bc_tag, bc_bufs)
                  for tn in range(TN)]
            for tn in range(TN):
                ln_norm(out, src_t, g_pm, b_pm, nm, tn, *bs[tn])
            if ps_pool is None:
                psLN.release()
            return out

        # ======== LN1 (out fp8 for DoubleRow QKV) ========
        xn1p = tc.alloc_tile_pool(name="xn1p", bufs=1, side="right")
        xn8 = layernorm(xT, ones_col_bf, BF16, g1_pm, bt1_pm, xn1p, FP8, "ln1")
        if stage == "ln1":
            dump_fm(xn8, 0)
        if rank < 1:
            xn1p.release()
            return nc

        # ======== QKV (fp8 DoubleRow) ========
        psB = tc.alloc_tile_pool(name="psB", bufs=2, space="PSUM")
        x2p = tc.alloc_tile_pool(name="x2p", bufs=1)
        x2T = x2p.tile([128, EC, P], BF16, tag="x2T", name="x2T_sb")
        qkvp = tc.alloc_tile_pool(name="qkvp", bufs=1)
        qT = qkvp.tile([128, EC, P], BF16, tag="qT", name="qT_sb")
        kT = qkvp.tile([128, EC, P], BF16, tag="kT", name="kT_sb")
        # v8[p, jcp, jpair, h, d]: token (2*jcp+jpair)*128+p, head h, d 0..64
        v8 = qkvp.tile([128, TC // 2, 2, H, 80], FP8, tag="v8", name="v8_sb")
        nc.vector.memset(v8[:, :, :, :, DH + 1:80], 0.0)
        nc.vector.memset(v8[:, :, :, :, DH], 1.0)

        def load_w8(dram, col0, nm, tag="w8", bufs=2):
            """[128, EC, 512] fp8 weight tile: contraction chunks x 512 out cols."""
            t = wpool.tile([128, EC, 512], FP8, tag=tag, bufs=bufs, name=nm)
            nc.sync.dma_start(
                t[:], dram[0:E, col0:col0 + 512].rearrange("(c p) n -> p c n", p=128))
            return t

        def emit_v(vg):
            wt = load_w8(wqkv_d, 2 * E + vg * 512, f"wv8_{vg}", tag="wv")
            for tcc in range(TC):
                ps = psB.tile([128, 512], F32, tag="mm", name=f"v_ps{vg}_{tcc}")
                nc.tensor.matmul(ps[:], ones_row[:, :128],
                                 bv_row[:, vg * 512:(vg + 1) * 512],
                                 start=True, stop=False)
                for p in range(EP):
                    nc.tensor.matmul(ps[:],
                                     xn8[:, 2 * p:2 * p + 2, tcc * 128:(tcc + 1) * 128],
                                     wt[:, 2 * p:2 * p + 2, :],
                                     start=False, stop=(p == EP - 1), perf_mode=DR)
                nc.vector.tensor_scalar_mul(
                    v8[:, tcc // 2, tcc % 2, vg * 8:(vg + 1) * 8, 0:DH],
                    ps[:].rearrange("p (h d) -> p h d", d=DH), 1.0 / WQ)

        def emit_qk(fg, fcls=range(4), wt=None):
            """fg 0,1 -> q chunks fg*4..fg*4+3; fg 2,3 -> k chunks."""
            if wt is None:
                wt = load_w8(wqkv_d, fg * 512, f"wqk8_{fg}")
            for fcl in fcls:
                fc = fg * 4 + fcl
                dst = qT if fc < EC else kT
                c = fc % EC
                for tn in range(TN):
                    sl = slice(tn * 512, (tn + 1) * 512)
                    ps = psB.tile([128, 512], F32, tag="mm", name=f"qk_ps{fc}_{tn}")
                    for p in range(EP):
                        nc.tensor.matmul(ps[:],
                                         wt[:, 2 * p:2 * p + 2, fcl * 128:(fcl + 1) * 128],
                                         xn8[:, 2 * p:2 * p + 2, sl],
                                         start=(p == 0), stop=(p == EP - 1),
                                         perf_mode=DR)
                    nc.vector.tensor_scalar(dst[:, c, sl], ps[:],
                                            1.0 / WQ, bqkv_pm[:, fc:fc + 1],
                                            ALU.mult, ALU.add)

        if stage == "qkv":
            emit_v(0)
            emit_v(1)
            emit_qk(0)
            emit_qk(2)
            emit_qk(1)
            emit_qk(3)
            dpool = tc.alloc_tile_pool(name="dumpq", bufs=2, side="right")
            for c in range(EC):
                for src, r0 in ((qT, 0), (kT, 1024)):
                    st = dpool.tile([128, P], F32, tag="dump", name=f"dq{r0}_{c}")
                    nc.scalar.activation(st[:], src[:, c, :], AF.Copy)
                    nc.sync.dma_start(dbg_d[r0 + c * 128: r0 + (c + 1) * 128, :], st[:])
            for tcc in range(TC):
                st = dpool.tile([128, H * DH], F32, tag="dump", name=f"dv_{tcc}")
                nc.vector.tensor_copy(st[:].rearrange("p (h d) -> p h d", d=DH),
                                      v8[:, tcc // 2, tcc % 2, :, 0:DH])
                nc.sync.dma_start(dbg_d[2048 + tcc * 128: 2048 + (tcc + 1) * 128, :],
                                  st[:])
            dpool.release()
        if rank < 2:
            psB.release()
            xn1p.release()
            qkvp.release()
            return nc

        # ======== attention ========
        # per (i-half, head-pair c): 8 j-chunk score MMs per head, row-tiled
        # (head 2c rows 0-63 @ tile (0,0), head 2c+1 rows 64-127 @ (64,0));
        # exp -> fp8 probs (x1/4); probs@V via 4 DoubleRow MMs over j-chunk
        # pairs with the ones column giving the denominator in row 64.
        psA = tc.alloc_tile_pool(name="psA", bufs=2, space="PSUM", side="right")
        attnp = tc.alloc_tile_pool(name="attnp", bufs=1, side="right")
        oT8 = attnp.tile([128, EC, P], FP8, tag="oT", name="oT_sb")
        wo8 = qkvp.tile([128, EC, E], FP8, tag="wo8", name="wo8_sb")
        nc.sync.dma_start(wo8[:], wo_d[0:E, :].rearrange("(c p) n -> p c n", p=128))

        def emit_scores_range(i, c, aT8, j0, j1):
            isl = slice(i * 512, (i + 1) * 512)
            for j in range(j0, j1):
                sps = psA.tile([128, 2, 512], F32, tag="sc", bufs=2,
                               name=f"s_ps{i}_{c}_{j}")
                jsl = slice(j * 128, (j + 1) * 128)
                nc.tensor.matmul(sps[:, 0, :], kT[0:DH, c, jsl], qT[0:DH, c, isl],
                                 start=True, stop=True)
                nc.tensor.matmul(sps[:, 1, :], kT[DH:128, c, jsl], qT[DH:128, c, isl],
                                 start=True, stop=True)
                nc.scalar.activation(aT8[:, j, :, :], sps[:], AF.Exp,
                                     bias=expb_sb[:], scale=SCALE)

        def emit_o(i, c, aT8):
            isl = slice(i * 512, (i + 1) * 512)
            for u in range(2):  # head within pair
                h = 2 * c + u
                ops = psA.tile([80, 512], F32, tag="o", bufs=2,
                               name=f"o_ps{i}_{c}_{u}")
                for g in range(TC // 2):
                    nc.tensor.matmul(ops[:], v8[:, g, :, h, :],
                                     aT8[:, 2 * g:2 * g + 2, u, :],
                                     start=(g == 0), stop=(g == TC // 2 - 1),
                                     perf_mode=DR)
                den_r = rows.tile([1, 512], F32, tag="den", bufs=3,
                                  name=f"den_{i}_{c}_{u}")
                nc.vector.tensor_copy(den_r[:], ops[DH:DH + 1, :])
                den_b = scr.tile([DH, 512], F32, tag="denb", bufs=2,
                                 name=f"denb_{i}_{c}_{u}")
                nc.gpsimd.partition_broadcast(den_b[:], den_r[:])
                rec = scr.tile([DH, 512], F32, tag="rec", bufs=2,
                               name=f"rec_{i}_{c}_{u}")
                nc.vector.reciprocal_approx_fast(rec[:], den_b[:])
                nc.vector.tensor_mul(oT8[u * DH:(u + 1) * DH, c, isl],
                                     ops[0:DH, :], rec[:])

        def emit_outproj(tn, fc):
            sl = slice(tn * 512, (tn + 1) * 512)
            ps = psB.tile([128, 512], F32, tag="mm", name=f"x2_ps{fc}_{tn}")
            for p in range(EP):
                nc.tensor.matmul(ps[:], wo8[:, 2 * p:2 * p + 2, fc * 128:(fc + 1) * 128],
                                 oT8[:, 2 * p:2 * p + 2, sl],
                                 start=(p == 0), stop=(p == EP - 1), perf_mode=DR)
            tmp = scr.tile([128, 512], F32, tag="og", bufs=2, name=f"x2t_{fc}_{tn}")
            nc.vector.tensor_scalar(tmp[:], ps[:], 1.0 / WOS, bo_pm[:, fc:fc + 1],
                                    ALU.mult, ALU.add)
            nc.vector.tensor_add(x2T[:, fc, sl], tmp[:], xT[:, fc, sl])

        # software pipeline: q/k chunks are produced per head-pair right
        # before their scores (softmax starts ~25us earlier); v lands during
        # the pair-0/1 exps; out-proj half-0 fills PE during i=1 softmax.
        from collections import deque

        qk_w = {}

        def qk_piece(fg, fcl):
            if fg not in qk_w:
                qk_w[fg] = load_w8(wqkv_d, fg * 512, f"wqk8_{fg}",
                                   tag="wqk", bufs=4)
            emit_qk(fg, fcls=[fcl], wt=qk_w[fg])

        fill1 = deque(
            [lambda fc=fc: emit_outproj(0, fc) for fc in range(EC)])
        pend = None
        for i in range(TN):
            for c in range(EC):
                if i == 0:
                    qk_piece(c // 4, c % 4)        # q chunk c
                    qk_piece(2 + c // 4, c % 4)    # k chunk c
                aT8 = attnp.tile([128, TC, 2, 512], FP8E5, tag="aT", bufs=2,
                                 name=f"aT_{i}_{c}")
                emit_scores_range(i, c, aT8, 0, 4)
                if i == 1 and c >= 1 and fill1:
                    fill1.popleft()()
                emit_scores_range(i, c, aT8, 4, TC)
                if i == 1 and c >= 1 and fill1:
                    fill1.popleft()()
                if i == 0 and c == 1:
                    emit_v(0)
                    emit_v(1)
                if pend is not None:
                    emit_o(*pend)
                pend = (i, c, aT8)
        emit_o(*pend)

        if stage == "attn":
            dump_fm(oT8, 0)
        if rank < 3:
            psA.release()
            psB.release()
            attnp.release()
            xn1p.release()
            qkvp.release()
            x2p.release()
            return nc

        # ======== out-proj tail (token-half 1) ========
        for fc in range(EC):
            emit_outproj(1, fc)
        psA.release()
        psB.release()
        attnp.release()
        xn1p.release()
        qkvp.release()
        if stage == "x2":
            dump_fm(x2T, 0)
        if rank < 4:
            x2p.release()
            return nc
        # ======== LN2 (two-phase) then FFN per token half ========
        psN = tc.alloc_tile_pool(name="psN", bufs=2, space="PSUM")
        xn2p = tc.alloc_tile_pool(name="xn2p", bufs=1, side="right")
        xn2 = layernorm(x2T, ones_col_bf, BF16, g2_pm, bt2_pm, xn2p, BF16,
                        "ln2", ps_pool=psN, st_tag="st", st_bufs=2,
                        bc_tag="h", bc_bufs=2)
        if stage == "ln2":
            dump_fm(xn2, 0)
        if rank < 9:
            xn2p.release()
            psN.release()
            x2p.release()
            return nc

        hp = tc.alloc_tile_pool(name="hp", bufs=1, side="right")
        for tn in range(TN):
            sl = slice(tn * 512, (tn + 1) * 512)
            hT = hp.tile([128, MC, 512], BF16, tag="hT", name=f"hT_{tn}")
            for fg in range(2):
                pcs = [None] * 4
                for mg in range(8):
                    if fg == 0:
                        w1t = wpool.tile([128, EC, 512], BF16, tag="w1", bufs=2,
                                         name=f"w1_{tn}_{mg}")
                        nc.sync.dma_start(
                            w1t[:], w1_d[0:E, mg * 512:(mg + 1) * 512]
                            .rearrange("(c p) n -> p c n", p=128))
                    for ml in range(4):
                        mc = mg * 4 + ml
                        if fg == 0:
                            hps = psN.tile([128, 512], F32, tag="h", bufs=2,
                                           name=f"h_ps{tn}_{mc}")
                            for c in range(EC):
                                nc.tensor.matmul(
                                    hps[:],
                                    w1t[:, c, ml * 128:(ml + 1) * 128],
                                    xn2[:, c, sl],
                                    start=(c == 0), stop=(c == EC - 1))
                            nc.scalar.activation(hT[:, mc, :], hps[:], AF.Gelu,
                                                 bias=b1_pm[:, mc:mc + 1])
                    for mpl in range(2):
                        mp = mg * 2 + mpl
                        w2t = wpool.tile([128, 2, 512], BF16, tag="w2s", bufs=4,
                                         name=f"w2_{tn}_{fg}_{mp}")
                        nc.sync.dma_start(
                            w2t[:], w2_d[2 * mp * 128:(2 * mp + 2) * 128,
                                         fg * 512:(fg + 1) * 512]
                            .rearrange("(u p) n -> p u n", p=128))
                        for fcl in range(4):
                            fc = fg * 4 + fcl
                            if pcs[fcl] is None:
                                pcs[fcl] = psN.tile([128, 512], F32, tag="acc",
                                                    bufs=4, name=f"ff_ps{tn}_{fc}")
                            for u in range(2):
                                nc.tensor.matmul(pcs[fcl][:],
                                                 w2t[:, u, fcl * 128:(fcl + 1) * 128],
                                                 hT[:, 2 * mp + u, :],
                                                 start=(mp == 0 and u == 0),
                                                 stop=(mp == MP - 1 and u == 1))
                for fcl in range(4):
                    fc = fg * 4 + fcl
                    og = scr.tile([128, 512], F32, tag="og", bufs=2,
                                  name=f"og_{tn}_{fc}")
                    nc.vector.scalar_tensor_tensor(og[:], pcs[fcl][:],
                                                   b2_pm[:, fc:fc + 1],
                                                   x2T[:, fc, sl], ALU.add, ALU.add)
                    nc.sync.dma_start(outT_d[fc * 128:(fc + 1) * 128, sl], og[:])
        hp.release()
        xn2p.release()
        psN.release()
        x2p.release()
    return nc  # noqa


def prep_inputs(x, ln1_g, ln1_b, wqkv, bqkv, wo, bo, ln2_g, ln2_b, w1, b1, w2, b2):
    """Host-side layout prep: shard x over batch, transpose to feature-major,
    fp8-quantize scaled weights, build partition-major bias/gamma tiles."""
    def pm(vec, nchunks):
        return np.ascontiguousarray(
            np.asarray(vec, dtype=np.float32).reshape(nchunks, 128).T)

    def fp8w(w, s):
        w = np.asarray(w, np.float32) * s
        return np.clip(w, -240.0, 240.0).astype(NP_FP8)

    bqkv = np.asarray(bqkv, np.float32)
    shared = dict(
        wqkv8=fp8w(wqkv, WQ),
        wo8=fp8w(wo, WOS),
        w1=np.asarray(w1, np.float32).astype(NP_BF16),
        w2=np.asarray(w2, np.float32).astype(NP_BF16),
        bv_row=(bqkv[2 * E:].reshape(1, E) * WQ).astype(NP_BF16),
        bo_pm=pm(bo, EC),
        b2_pm=pm(b2, EC),
        bqkv_pm=pm(bqkv[:2 * E], 16),
        b1_pm=pm(b1, MC),
        g1_pm=pm(ln1_g, EC),
        bt1_pm=pm(ln1_b, EC),
        g2_pm=pm(ln2_g, EC),
        bt2_pm=pm(ln2_b, EC),
        ones_row=np.ones((1, 512), np.float32).astype(NP_BF16),
        ones_col=np.ones((128, 1), np.float32),
        ones_col_bf=np.ones((128, 1), np.float32).astype(NP_BF16),
    )
    x = np.asarray(x, np.float32)
    in_maps = []
    for b in range(B):
        m = dict(shared)
        m["xT"] = np.ascontiguousarray(x[b, :, :E].T).astype(NP_BF16)
        in_maps.append(m)
    return in_maps


_CACHE = {}


def run_on_hw(inputs, stage="full", trace=False, **trace_kw):
    key = stage
    if key not in _CACHE:
        nc = build_program(stage)
        nc.compile()
        _CACHE[key] = nc
    nc = _CACHE[key]
    in_maps = prep_inputs(**inputs)
    res = run_bass_kernel_spmd(nc, in_maps, list(range(NCORES)), trace=trace,
                               **trace_kw)
    return res


def kernel(**inputs) -> np.ndarray:
    res = run_on_hw(inputs, stage="full", trace=False)
    out = np.zeros((B, P, E + 1), np.float32)
    for b in range(B):
        out[b, :, :E] = res.results[b]["outT"].T
    return out
